# revision 1
# baseline (speedup 1.0000x reference)
"""ChebyASPIRE spectral filter on 8 TRN2 NeuronCores.

Algorithm (Gram-matrix formulation):
  phase 1: Z = X^T X  (4096x4096), column-sharded: core i computes
           Z[:, i*512:(i+1)*512] from a full stream of X.  Operands are
           fp8-e4m3 (X pre-scaled by 32 on host) using DoubleRow perf
           mode (2 fp8 macs/cell/cycle); PSUM accumulates fp32; the
           1/1024 descale folds into the PSUM->SBUF copy.  Z kept fp16.
  phase 2: Chebyshev recurrence t_k = 2*Zs t_{k-1} - t_{k-2} with
           Zs = (Z - t_mid I)/t_half, applied to V = R^T (4096x256).
           Row-sharded: core i computes rows [i*512, (i+1)*512) of each
           t_k using lhsT = Z[:, ib] (== Z[ib, :]^T by symmetry), then
           AllGathers the new t shard (fp8, |t| <= 1) so every core has
           the full t for the next step.  Local recurrence state and
           accumulator stay fp32.  The batch is split into two halves
           that alternate on the PE so each half's AllGather+reload
           chain hides under the other half's matmuls.

All DMA-heavy tensors are pre-blocked on the host into partition-major
layouts so every DMA descriptor line is 512B-4KB contiguous: DMA trigger
issue and descriptor generation (not wire bytes) dominate otherwise.

Inputs come in full; sharding/layout prep happens on host.  Scalars
(t_mid, t_half, coeffs) are baked into the program as immediates; the
program is rebuilt (and NEFF-cached) per distinct scalar set.
"""
import sys

sys.path.insert(0, "/opt/trn_rl_repo")

import numpy as np

M, N, B = 8192, 4096, 256
NC = 8
CB = N // NC          # 512 columns/rows per core
DEG = 20              # Chebyshev degree (21 coeffs)
KT1 = M // 128        # 64 k-tiles in phase 1
KP1 = KT1 // 2        # 32 DoubleRow k-pairs in phase 1
MP1 = N // 128        # 32 m-passes in phase 1
KT2 = N // 128        # 32 k-tiles in phase 2
MS2 = CB // 128       # 4 m-subs in phase 2
NH = 2                # batch halves in phase 2
BH = B // NH          # 128 columns per half
XSCALE = 32.0         # host-side fp8 pre-scale on X
ZDESCALE = 1.0 / (XSCALE * XSCALE)

_BUILD_CACHE = {}


def _build(scalars):
    """Build the SPMD Bass program for the given (t_mid, t_half, *coeffs)."""
    from concourse import bacc, tile, mybir

    tm, th = scalars[0], scalars[1]
    c = scalars[2:]
    f8 = mybir.dt.float8e4
    f16 = mybir.dt.float16
    f32 = mybir.dt.float32
    mult = mybir.AluOpType.mult
    add = mybir.AluOpType.add
    sub = mybir.AluOpType.subtract
    DR = mybir.MatmulPerfMode.DoubleRow

    nc = bacc.Bacc("TRN2", target_bir_lowering=False, debug=False,
                   num_devices=NC)
    # host-blocked layouts (see _run): lines are contiguous per partition
    XL = nc.dram_tensor("XL8", [MP1 * 128 * KT1, 128], f8,
                        kind="ExternalInput")      # [mp, p, kk, mc]
    XR = nc.dram_tensor("XR8", [128 * KT1, CB], f8,
                        kind="ExternalInput")      # [p, kk, cb]
    VL = nc.dram_tensor("VL8", [NH * 128 * KT2, BH], f8,
                        kind="ExternalInput")      # [h, p, kk, b]
    Vb = nc.dram_tensor("Vblk32", [CB, B], f32, kind="ExternalInput")
    acc_out = nc.dram_tensor("acc_out", [CB, B], f32, kind="ExternalOutput")

    RCH = 8                      # k-tiles per rhs_res chunk (phase 1)
    LCH = 16                     # k-tiles per lh chunk (phase 1)
    XR3 = XR[:, :].rearrange("(p kk) cb -> p kk cb", p=128)
    with tile.TileContext(nc) as tc:
        with (
            tc.tile_pool(name="persist", bufs=1) as persist,
            tc.tile_pool(name="lstream", bufs=2) as lstream,
            tc.tile_pool(name="rhsp", bufs=2) as rhsp,
            tc.tile_pool(name="dve", bufs=4) as dvep,
            tc.tile_pool(name="stagep", bufs=2) as stagep,
            tc.tile_pool(name="ps1", bufs=2, space="PSUM") as ps1,
            tc.tile_pool(name="ps2", bufs=6, space="PSUM") as ps2p,
            tc.tile_pool(name="dram", bufs=1, space="DRAM") as dram,
        ):
            # ---------------- phase 1: Z[:, ib] = X^T X[:, ib] -------------
            # resident rhs X[:, ib], chunked so matmuls start early
            rhs_res = [persist.tile([128, RCH, CB], f8, name=f"rhs_res{cc}")
                       for cc in range(KT1 // RCH)]
            for cc in range(KT1 // RCH):
                nc.sync.dma_start(rhs_res[cc][:],
                                  XR3[:, cc * RCH:(cc + 1) * RCH, :])

            # phase-2 state init (consumed by DVE only, so these early
            # loads cannot pull matmuls into phase 1)
            tstate = [[persist.tile([128, MS2, BH], f32, name=f"tst{h}_{i}")
                       for i in range(3)] for h in range(NH)]
            acc = [persist.tile([128, MS2, BH], f32, name=f"acc{h}")
                   for h in range(NH)]
            zero = persist.tile([128, BH], f32, name="zero")
            nc.any.memset(zero[:], 0.0)
            Vb3 = Vb[:, :].rearrange("(ms p) b -> p ms b", p=128)
            for h in range(NH):
                nc.sync.dma_start(tstate[h][0][:],
                                  Vb3[:, :, h * BH:(h + 1) * BH])

            # warm-up AllGather: burns the first-collective cold cost
            # concurrently with phase-1 compute.
            agin_w = dram.tile([128, MS2 * BH], f8, name="agin_w")
            agout_w = dram.tile([NC * 128, MS2 * BH], f8,
                                addr_space="Shared", name="agout_w")
            nc.gpsimd.collective_compute(
                "AllGather", mybir.AluOpType.bypass,
                replica_groups=[list(range(NC))],
                ins=[agin_w[:]], outs=[agout_w[:]])

            zk = [persist.tile([128, CB], f16, name=f"zk{i}")
                  for i in range(KT2)]

            for mp in range(MP1):
                lhs = [lstream.tile([128, LCH, 128], f8, name=f"lh{h}")
                       for h in range(KT1 // LCH)]
                Xm3 = (XL[mp * 128 * KT1:(mp + 1) * 128 * KT1, :]
                       .rearrange("(p kk) mc -> p kk mc", p=128))
                # lhs triggers on the scalar DGE queue: off the sync
                # queue, and the s==1 rh loads queue up behind them so
                # step-1 matmuls can't be scheduled into phase 1.
                for h in range(KT1 // LCH):
                    nc.scalar.dma_start(lhs[h][:],
                                        Xm3[:, h * LCH:(h + 1) * LCH, :])
                zps = ps1.tile([128, CB], f32, name="zps")
                for kp in range(KP1):
                    kk = 2 * kp
                    nc.tensor.matmul(
                        zps[:],
                        lhs[kk // LCH][:, kk % LCH:kk % LCH + 2, :],
                        rhs_res[kk // RCH][:, kk % RCH:kk % RCH + 2, :],
                        start=(kp == 0), stop=(kp == KP1 - 1),
                        perf_mode=DR)
                nc.vector.tensor_scalar_mul(zk[mp][:], zps[:], ZDESCALE)

            # ---------------- phase 2: Chebyshev recurrence ----------------
            agout = [[None] * NH for _ in range(DEG)]

            for s in range(1, DEG + 1):
                for h in range(NH):
                    # rhs: full t_{s-1} half (4096 x 128) fp8.
                    # SBUF tile [p, kk, b]; global t row = kk*128+p with
                    # kk = 4*rank + ms, matching the blocked agout layout
                    # [rank, p, ms*b] (and VL's [h, p, kk, b]).
                    rh = rhsp.tile([128, NC, MS2, BH], f8, name=f"rh{h}")
                    if s == 1:
                        src = (VL[h * 128 * KT2:(h + 1) * 128 * KT2, :]
                               .rearrange("(p r ms) b -> p r ms b",
                                          p=128, r=NC))
                        nc.scalar.dma_start(rh[:], src[:])
                    else:
                        src = (agout[s - 2][h][:, :]
                               .rearrange("(r p) (ms b) -> p r ms b",
                                          p=128, b=BH))
                        nc.scalar.dma_start(rh[:], src[:])

                    Tc = tstate[h][(s - 1) % 3]
                    Tp = tstate[h][(s - 2) % 3] if s >= 2 else None
                    Tn = tstate[h][s % 3]
                    ach = acc[h]
                    if s < DEG:
                        stage = stagep.tile([128, MS2, BH], f8,
                                            name=f"stage{h}")
                        agin = dram.tile([128, MS2 * BH], f8,
                                         name=f"agin{s}_{h}")
                        agin3 = agin[:, :].rearrange("p (ms b) -> p ms b",
                                                     b=BH)

                    for ms in range(MS2):
                        wps = ps2p.tile([128, BH], f32, name="wps")
                        for kk in range(KT2):
                            nc.tensor.matmul(
                                wps[:],
                                zk[kk][:, ms * 128:(ms + 1) * 128],
                                rh[:, kk // MS2, kk % MS2, :],
                                start=(kk == 0), stop=(kk == KT2 - 1))

                        u = dvep.tile([128, BH], f32, name="u")
                        # u = W - tm * Tc
                        nc.vector.scalar_tensor_tensor(
                            u[:], Tc[:, ms, :], -tm, wps[:],
                            op0=mult, op1=add)
                        if s == 1:
                            # T1 = u / th ;  acc = c0*V + c1*T1
                            nc.vector.scalar_tensor_tensor(
                                Tn[:, ms, :], u[:], 1.0 / th, zero[:],
                                op0=mult, op1=sub)
                            nc.vector.tensor_scalar_mul(
                                ach[:, ms, :], Tc[:, ms, :], c[0])
                            nc.vector.scalar_tensor_tensor(
                                ach[:, ms, :], Tn[:, ms, :], c[1],
                                ach[:, ms, :], op0=mult, op1=add)
                        else:
                            # Tn = (2/th)*u - Tp ; acc += c_s * Tn
                            nc.vector.scalar_tensor_tensor(
                                Tn[:, ms, :], u[:], 2.0 / th, Tp[:, ms, :],
                                op0=mult, op1=sub)
                        if s < DEG:
                            nc.vector.tensor_copy(stage[:, ms, :],
                                                  Tn[:, ms, :])
                        if s > 1:
                            nc.vector.scalar_tensor_tensor(
                                ach[:, ms, :], Tn[:, ms, :], c[s],
                                ach[:, ms, :], op0=mult, op1=add)

                    if s < DEG:
                        nc.sync.dma_start(agin3[:], stage[:])
                        agout[s - 1][h] = dram.tile(
                            [NC * 128, MS2 * BH], f8, addr_space="Shared",
                            name=f"agout{s}_{h}")
                        nc.gpsimd.collective_compute(
                            "AllGather",
                            mybir.AluOpType.bypass,
                            replica_groups=[list(range(NC))],
                            ins=[agin[:]],
                            outs=[agout[s - 1][h][:]],
                        )

            out3 = acc_out[:, :].rearrange("(ms p) b -> p ms b", p=128)
            for h in range(NH):
                nc.sync.dma_start(out3[:, :, h * BH:(h + 1) * BH],
                                  acc[h][:])

    nc.finalize()
    return nc


def _get_program(scalars):
    key = tuple(np.asarray(scalars, np.float64).tolist())
    if key not in _BUILD_CACHE:
        _BUILD_CACHE[key] = _build(key)
    return _BUILD_CACHE[key]


def _run(X, R, coeffs, t_mid, t_half, trace=False):
    import ml_dtypes
    from concourse.bass_utils import run_bass_kernel_spmd

    X = np.ascontiguousarray(np.asarray(X, np.float32))
    R = np.ascontiguousarray(np.asarray(R, np.float32))
    coeffs = np.asarray(coeffs, np.float32)
    tm = float(np.asarray(t_mid).reshape(-1)[0])
    th = float(np.asarray(t_half).reshape(-1)[0])

    nc = _get_program((tm, th, *[float(v) for v in coeffs]))

    f8np = ml_dtypes.float8_e4m3
    X8 = (X * XSCALE).astype(f8np)
    V32 = np.ascontiguousarray(R.T.astype(np.float32))   # (N, B)
    V8 = V32.astype(f8np)

    # blocked lhs stream [mp, p, kk, mc]: contiguous 2KB lines per (p)
    XL8 = np.ascontiguousarray(
        X8.reshape(KT1, 128, MP1, 128).transpose(2, 1, 0, 3)
    ).reshape(MP1 * 128 * KT1, 128)
    # blocked step-1 rhs [h, p, kk, b]
    VL8 = np.ascontiguousarray(
        V8.reshape(KT2, 128, NH, BH).transpose(2, 1, 0, 3)
    ).reshape(NH * 128 * KT2, BH)

    in_maps = []
    for i in range(NC):
        ib = slice(i * CB, (i + 1) * CB)
        Xb8 = X8[:, ib]
        # blocked resident rhs [p, kk, cb]: 4KB lines per (p, chunk)
        XR8 = np.ascontiguousarray(
            Xb8.reshape(KT1, 128, CB).transpose(1, 0, 2)
        ).reshape(128 * KT1, CB)
        in_maps.append({
            "XL8": XL8,
            "XR8": XR8,
            "VL8": VL8,
            "Vblk32": np.ascontiguousarray(V32[ib, :]),
        })

    res = run_bass_kernel_spmd(nc, in_maps, core_ids=list(range(NC)),
                               trace=trace)

    out = np.empty((B, N), np.float32)
    for i in range(NC):
        out[:, i * CB:(i + 1) * CB] = res.results[i]["acc_out"].T
    return out, res


def kernel(X, R, coeffs, t_mid, t_half):
    out, _ = _run(X, R, coeffs, t_mid, t_half, trace=False)
    return out



# revision 5
# speedup vs baseline: 1.6364x; 1.6364x over previous
"""ChebyASPIRE spectral filter on 8 TRN2 NeuronCores.

Algorithm (Gram + spectral deflation):
  host prep: Z = X^T X has one dominant eigenpair (lam1, u) -- X is
           iid-sparse-random so Z = strong rank-1 + tight bulk whose
           edge lam2 << lam1.  Host power-iteration (on the fp8-exact
           replica of the device Z) gives (lam1, u) in a handful of
           iterations plus a deflated-power estimate of lam2.  The
           degree-20 Chebyshev filter p restricted to the bulk interval
           [(0-tm)/th, (lam2*SAFETY-tm)/th] is refit with a low-degree
           Chebyshev q (degree adapted to fit error; ~5 here), and the
           deflated u-direction is patched exactly with a rank-1 term
           beta*u*w^T, beta = p(s1) - q(nu), w = u^T V.
  phase 1: Z_defl[:, ib] = X^T X[:, ib] - lam1 u u[ib]^T, column-
           sharded.  fp8 DoubleRow matmuls for X^T X (X pre-scaled by
           32), one extra fp16 rank-1 matmul per 128-row chunk folds
           the deflation into the same PSUM accumulation.  Z_defl fp16.
  phase 2: Chebyshev recurrence on the rescaled operator
           Yhat = (Z_defl - tm' I)/th' (tm' = tm + th*mhat,
           th' = th*hhat) with only deg(q) steps; identical loop
           structure to the degree-20 version: row-sharded matmuls,
           fp8 AllGather of the new t-shard per step, two batch halves
           alternating on the PE to hide the AllGather.  Finally
           acc += u[ib] (beta w)^T via tiny k=1 matmuls.

All DMA-heavy tensors are pre-blocked on the host into partition-major
layouts so every DMA descriptor line is 512B-4KB contiguous.

Inputs come in full; sharding/layout prep happens on host.  Scalars
(tm', th', q coeffs) are baked into the program as immediates; the
program is rebuilt (and NEFF-cached) per distinct scalar set.
"""
import sys

sys.path.insert(0, "/opt/trn_rl_repo")

import numpy as np

M, N, B = 8192, 4096, 256
NC = 8
CB = N // NC          # 512 columns/rows per core
KT1 = M // 128        # 64 k-tiles in phase 1
KP1 = KT1 // 2        # 32 DoubleRow k-pairs in phase 1
MP1 = N // 128        # 32 m-passes in phase 1
KT2 = N // 128        # 32 k-tiles in phase 2
MS2 = CB // 128       # 4 m-subs in phase 2
NH = 2                # batch halves in phase 2
BH = B // NH          # 128 columns per half
XSCALE = 32.0         # host-side fp8 pre-scale on X
ZDESCALE = 1.0 / (XSCALE * XSCALE)

_BUILD_CACHE = {}


def _build(key):
    """Build the SPMD Bass program for (tm', th', deg, *q coeffs)."""
    from concourse import bacc, tile, mybir

    tm, th, deg = key[0], key[1], int(key[2])
    c = key[3:]
    assert len(c) == deg + 1
    f8 = mybir.dt.float8e4
    f16 = mybir.dt.float16
    f32 = mybir.dt.float32
    mult = mybir.AluOpType.mult
    add = mybir.AluOpType.add
    sub = mybir.AluOpType.subtract
    DR = mybir.MatmulPerfMode.DoubleRow

    nc = bacc.Bacc("TRN2", target_bir_lowering=False, debug=False,
                   num_devices=NC)
    # host-blocked layouts (see _run): lines are contiguous per partition
    XL = nc.dram_tensor("XL8", [MP1 * 128 * KT1, 128], f8,
                        kind="ExternalInput")      # [mp, p, kk, mc]
    XR = nc.dram_tensor("XR8", [128 * KT1, CB], f8,
                        kind="ExternalInput")      # [p, kk, cb]
    VL = nc.dram_tensor("VL8", [NH * 128 * KT2, BH], f8,
                        kind="ExternalInput")      # [h, p, kk, b]
    Vb = nc.dram_tensor("Vblk32", [CB, B], f32, kind="ExternalInput")
    # deflation rank-1 operands: UL = -s*u (lhs layout), UR = s*u[ib]
    UL = nc.dram_tensor("UL16", [1, N], f16, kind="ExternalInput")
    UR = nc.dram_tensor("UR16", [1, CB], f16, kind="ExternalInput")
    # final rank-1 patch: U16 = u[ib], BW16 = beta*w
    U16 = nc.dram_tensor("U16", [1, CB], f16, kind="ExternalInput")
    BW = nc.dram_tensor("BW16", [1, B], f16, kind="ExternalInput")
    acc_out = nc.dram_tensor("acc_out", [CB, B], f32, kind="ExternalOutput")

    RCH = 8                      # k-tiles per rhs_res chunk (phase 1)
    LCH = 16                     # k-tiles per lh chunk (phase 1)
    XR3 = XR[:, :].rearrange("(p kk) cb -> p kk cb", p=128)
    with tile.TileContext(nc) as tc:
        with (
            tc.tile_pool(name="persist", bufs=1) as persist,
            tc.tile_pool(name="lstream", bufs=2) as lstream,
            tc.tile_pool(name="rhsp", bufs=2) as rhsp,
            tc.tile_pool(name="dve", bufs=4) as dvep,
            tc.tile_pool(name="stagep", bufs=2) as stagep,
            tc.tile_pool(name="ps1", bufs=2, space="PSUM") as ps1,
            tc.tile_pool(name="ps2", bufs=6, space="PSUM") as ps2p,
            tc.tile_pool(name="dram", bufs=1, space="DRAM") as dram,
        ):
            # ---------------- phase 1: Z_defl[:, ib] -----------------------
            # resident rhs X[:, ib], chunked so matmuls start early
            rhs_res = [persist.tile([128, RCH, CB], f8, name=f"rhs_res{cc}")
                       for cc in range(KT1 // RCH)]
            for cc in range(KT1 // RCH):
                nc.sync.dma_start(rhs_res[cc][:],
                                  XR3[:, cc * RCH:(cc + 1) * RCH, :])
            ul = persist.tile([1, N], f16, name="ul")
            ur = persist.tile([1, CB], f16, name="ur")
            nc.sync.dma_start(ul[:], UL[:, :])
            nc.sync.dma_start(ur[:], UR[:, :])

            # phase-2 state init (consumed by DVE only, so these early
            # loads cannot pull matmuls into phase 1)
            tstate = [[persist.tile([128, MS2, BH], f32, name=f"tst{h}_{i}")
                       for i in range(3)] for h in range(NH)]
            acc = [persist.tile([128, MS2, BH], f32, name=f"acc{h}")
                   for h in range(NH)]
            zero = persist.tile([128, BH], f32, name="zero")
            nc.any.memset(zero[:], 0.0)
            Vb3 = Vb[:, :].rearrange("(ms p) b -> p ms b", p=128)
            for h in range(NH):
                nc.sync.dma_start(tstate[h][0][:],
                                  Vb3[:, :, h * BH:(h + 1) * BH])
            u16 = persist.tile([1, CB], f16, name="u16")
            bw = persist.tile([1, B], f16, name="bw")
            nc.sync.dma_start(u16[:], U16[:, :])
            nc.sync.dma_start(bw[:], BW[:, :])

            # warm-up AllGather: burns the first-collective cold cost
            # concurrently with phase-1 compute.
            agin_w = dram.tile([128, MS2 * BH], f8, name="agin_w")
            agout_w = dram.tile([NC * 128, MS2 * BH], f8,
                                addr_space="Shared", name="agout_w")
            nc.gpsimd.collective_compute(
                "AllGather", mybir.AluOpType.bypass,
                replica_groups=[list(range(NC))],
                ins=[agin_w[:]], outs=[agout_w[:]])

            zk = [persist.tile([128, CB], f16, name=f"zk{i}")
                  for i in range(KT2)]

            for mp in range(MP1):
                lhs = [lstream.tile([128, LCH, 128], f8, name=f"lh{h}")
                       for h in range(KT1 // LCH)]
                Xm3 = (XL[mp * 128 * KT1:(mp + 1) * 128 * KT1, :]
                       .rearrange("(p kk) mc -> p kk mc", p=128))
                # lhs triggers on the scalar DGE queue: off the sync
                # queue, and the s==1 rh loads queue up behind them so
                # step-1 matmuls can't be scheduled into phase 1.
                for h in range(KT1 // LCH):
                    nc.scalar.dma_start(lhs[h][:],
                                        Xm3[:, h * LCH:(h + 1) * LCH, :])
                zps = ps1.tile([128, CB], f32, name="zps")
                # deflation: PSUM starts at -1024*lam1*u[mp-chunk]*u[ib]^T
                nc.tensor.matmul(
                    zps[:],
                    ul[:, mp * 128:(mp + 1) * 128],
                    ur[:, :],
                    start=True, stop=False)
                for kp in range(KP1):
                    kk = 2 * kp
                    nc.tensor.matmul(
                        zps[:],
                        lhs[kk // LCH][:, kk % LCH:kk % LCH + 2, :],
                        rhs_res[kk // RCH][:, kk % RCH:kk % RCH + 2, :],
                        start=False, stop=(kp == KP1 - 1),
                        perf_mode=DR)
                nc.vector.tensor_scalar_mul(zk[mp][:], zps[:], ZDESCALE)

            # ---------------- phase 2: Chebyshev recurrence ----------------
            agout = [[None] * NH for _ in range(deg)]

            for s in range(1, deg + 1):
                for h in range(NH):
                    # rhs: full t_{s-1} half (4096 x 128) fp8.
                    # SBUF tile [p, kk, b]; global t row = kk*128+p with
                    # kk = 4*rank + ms, matching the blocked agout layout
                    # [rank, p, ms*b] (and VL's [h, p, kk, b]).
                    rh = rhsp.tile([128, NC, MS2, BH], f8, name=f"rh{h}")
                    if s == 1:
                        src = (VL[h * 128 * KT2:(h + 1) * 128 * KT2, :]
                               .rearrange("(p r ms) b -> p r ms b",
                                          p=128, r=NC))
                        nc.scalar.dma_start(rh[:], src[:])
                    else:
                        src = (agout[s - 2][h][:, :]
                               .rearrange("(r p) (ms b) -> p r ms b",
                                          p=128, b=BH))
                        nc.scalar.dma_start(rh[:], src[:])

                    Tc = tstate[h][(s - 1) % 3]
                    Tp = tstate[h][(s - 2) % 3] if s >= 2 else None
                    Tn = tstate[h][s % 3]
                    ach = acc[h]
                    if s < deg:
                        stage = stagep.tile([128, MS2, BH], f8,
                                            name=f"stage{h}")
                        agin = dram.tile([128, MS2 * BH], f8,
                                         name=f"agin{s}_{h}")
                        agin3 = agin[:, :].rearrange("p (ms b) -> p ms b",
                                                     b=BH)

                    for ms in range(MS2):
                        wps = ps2p.tile([128, BH], f32, name="wps")
                        for kk in range(KT2):
                            nc.tensor.matmul(
                                wps[:],
                                zk[kk][:, ms * 128:(ms + 1) * 128],
                                rh[:, kk // MS2, kk % MS2, :],
                                start=(kk == 0), stop=(kk == KT2 - 1))

                        u = dvep.tile([128, BH], f32, name="u")
                        # u = W - tm' * Tc
                        nc.vector.scalar_tensor_tensor(
                            u[:], Tc[:, ms, :], -tm, wps[:],
                            op0=mult, op1=add)
                        if s == 1:
                            # T1 = u / th' ;  acc = q0*V + q1*T1
                            nc.vector.scalar_tensor_tensor(
                                Tn[:, ms, :], u[:], 1.0 / th, zero[:],
                                op0=mult, op1=sub)
                            nc.vector.tensor_scalar_mul(
                                ach[:, ms, :], Tc[:, ms, :], c[0])
                            nc.vector.scalar_tensor_tensor(
                                ach[:, ms, :], Tn[:, ms, :], c[1],
                                ach[:, ms, :], op0=mult, op1=add)
                        else:
                            # Tn = (2/th')*u - Tp ; acc += q_s * Tn
                            nc.vector.scalar_tensor_tensor(
                                Tn[:, ms, :], u[:], 2.0 / th, Tp[:, ms, :],
                                op0=mult, op1=sub)
                        if s < deg:
                            nc.vector.tensor_copy(stage[:, ms, :],
                                                  Tn[:, ms, :])
                        if s > 1:
                            nc.vector.scalar_tensor_tensor(
                                ach[:, ms, :], Tn[:, ms, :], c[s],
                                ach[:, ms, :], op0=mult, op1=add)

                    if s < deg:
                        nc.sync.dma_start(agin3[:], stage[:])
                        agout[s - 1][h] = dram.tile(
                            [NC * 128, MS2 * BH], f8, addr_space="Shared",
                            name=f"agout{s}_{h}")
                        nc.gpsimd.collective_compute(
                            "AllGather",
                            mybir.AluOpType.bypass,
                            replica_groups=[list(range(NC))],
                            ins=[agin[:]],
                            outs=[agout[s - 1][h][:]],
                        )

            # rank-1 patch: acc[h][:, ms, :] += u[ib ms-chunk] (beta w_h)^T
            out3 = acc_out[:, :].rearrange("(ms p) b -> p ms b", p=128)
            for h in range(NH):
                for ms in range(MS2):
                    pr1 = ps2p.tile([128, BH], f32, name="wps")
                    nc.tensor.matmul(
                        pr1[:],
                        u16[:, ms * 128:(ms + 1) * 128],
                        bw[:, h * BH:(h + 1) * BH],
                        start=True, stop=True)
                    nc.vector.scalar_tensor_tensor(
                        acc[h][:, ms, :], pr1[:], 1.0, acc[h][:, ms, :],
                        op0=mult, op1=add)
                nc.sync.dma_start(out3[:, :, h * BH:(h + 1) * BH],
                                  acc[h][:])

    nc.finalize()
    return nc


def _get_program(key):
    key = tuple(np.asarray(key, np.float64).tolist())
    if key not in _BUILD_CACHE:
        _BUILD_CACHE[key] = _build(key)
    return _BUILD_CACHE[key]


def _spectral_prep(X8f, R, coeffs, tm, th):
    """Host-side: eigen structure of the device Z + low-degree refit.

    Returns (key, u32, beta*w) where key = (tm', th', deg, *q).
    """
    N_ = X8f.shape[1]

    def zmv(v):
        return (X8f.T @ (X8f @ v)) * ZDESCALE

    rng = np.random.default_rng(1)
    v = rng.standard_normal(N_).astype(np.float32)
    v /= np.linalg.norm(v)
    lam1 = 0.0
    for _ in range(12):
        w_ = zmv(v)
        lam1 = float(np.linalg.norm(w_))
        v = w_ / lam1
    u = v.astype(np.float64)
    u /= np.linalg.norm(u)

    v2 = rng.standard_normal(N_).astype(np.float32)
    v2 -= (u @ v2).astype(np.float32) * u.astype(np.float32)
    v2 /= np.linalg.norm(v2)
    lam2 = 0.0
    for _ in range(12):
        w_ = zmv(v2)
        w_ -= (u @ w_).astype(np.float32) * u.astype(np.float32)
        lam2 = float(np.linalg.norm(w_))
        v2 = w_ / lam2

    co = np.asarray(coeffs, np.float64)
    DEG0 = len(co) - 1

    def p_eval(x):
        x = np.asarray(x, np.float64)
        t0 = np.ones_like(x)
        t1 = x
        s = co[0] * t0 + co[1] * t1
        for k in range(2, DEG0 + 1):
            t0, t1 = t1, 2 * x * t1 - t0
            s += co[k] * t1
        return s

    s1 = (lam1 - tm) / th
    nu = (0.0 - tm) / th
    SAFETY = 1.35
    hi = (lam2 * SAFETY - tm) / th
    lo = nu
    split_ok = (lam2 * SAFETY < 0.6 * lam1) and hi > lo
    if not split_ok:
        # no spectral gap: fall back to the full interval, degree 20
        lo = nu
        hi = (lam1 * 1.01 - tm) / th
        s1 = hi  # beta -> ~0

    def cheb_fit(lo_, hi_, d_):
        j = np.arange(d_ + 1)
        theta = np.pi * (j + 0.5) / (d_ + 1)
        xn = (lo_ + hi_) / 2 + (hi_ - lo_) / 2 * np.cos(theta)
        fn = p_eval(xn)
        q_ = np.array([2.0 / (d_ + 1) * np.sum(fn * np.cos(k * theta))
                       for k in range(d_ + 1)])
        q_[0] /= 2
        return q_

    xs = np.linspace(lo, hi, 2001)
    deg = DEG0
    for d_ in range(3, DEG0 + 1):
        q = cheb_fit(lo, hi, d_)
        y = (xs - (lo + hi) / 2) / ((hi - lo) / 2)
        t0 = np.ones_like(y)
        t1 = y
        sfit = q[0] * t0 + q[1] * t1
        for k in range(2, d_ + 1):
            t0, t1 = t1, 2 * y * t1 - t0
            sfit += q[k] * t1
        if np.abs(sfit - p_eval(xs)).max() < 2e-5:
            deg = d_
            break
    q = cheb_fit(lo, hi, deg)

    mhat = (lo + hi) / 2
    hhat = (hi - lo) / 2
    tmp = tm + th * mhat
    thp = th * hhat

    # q-sum at the mapped u-eigenvalue (deflated Z has u-eig 0 -> nu)
    ynu = (nu - mhat) / hhat
    t0, t1 = 1.0, ynu
    qnu = q[0] + q[1] * t1
    for k in range(2, deg + 1):
        t0, t1 = t1, 2 * ynu * t1 - t0
        qnu += q[k] * t1
    beta = p_eval(s1) - qnu if split_ok else 0.0

    w = u @ R.T.astype(np.float64)          # (B,)
    key = (tmp, thp, float(deg)) + tuple(q.tolist())
    return key, lam1, u, beta * w, split_ok


def _run(X, R, coeffs, t_mid, t_half, trace=False):
    import ml_dtypes
    from concourse.bass_utils import run_bass_kernel_spmd

    X = np.ascontiguousarray(np.asarray(X, np.float32))
    R = np.ascontiguousarray(np.asarray(R, np.float32))
    coeffs = np.asarray(coeffs, np.float32)
    tm = float(np.asarray(t_mid).reshape(-1)[0])
    th = float(np.asarray(t_half).reshape(-1)[0])

    f8np = ml_dtypes.float8_e4m3
    X8 = (X * XSCALE).astype(f8np)
    X8f = X8.astype(np.float32)
    key, lam1, u, bw, split_ok = _spectral_prep(X8f, R, coeffs, tm, th)

    nc = _get_program(key)

    V32 = np.ascontiguousarray(R.T.astype(np.float32))   # (N, B)
    V8 = V32.astype(f8np)

    # blocked lhs stream [mp, p, kk, mc]: contiguous 2KB lines per (p)
    XL8 = np.ascontiguousarray(
        X8.reshape(KT1, 128, MP1, 128).transpose(2, 1, 0, 3)
    ).reshape(MP1 * 128 * KT1, 128)
    # blocked step-1 rhs [h, p, kk, b]
    VL8 = np.ascontiguousarray(
        V8.reshape(KT2, 128, NH, BH).transpose(2, 1, 0, 3)
    ).reshape(NH * 128 * KT2, BH)

    sdefl = np.sqrt(1024.0 * lam1) if split_ok else 0.0
    u32 = u.astype(np.float32)
    UL16 = np.ascontiguousarray((-sdefl * u32)[None, :]).astype(np.float16)
    BW16 = np.ascontiguousarray(bw.astype(np.float16)[None, :])

    in_maps = []
    for i in range(NC):
        ib = slice(i * CB, (i + 1) * CB)
        Xb8 = X8[:, ib]
        # blocked resident rhs [p, kk, cb]: 4KB lines per (p, chunk)
        XR8 = np.ascontiguousarray(
            Xb8.reshape(KT1, 128, CB).transpose(1, 0, 2)
        ).reshape(128 * KT1, CB)
        in_maps.append({
            "XL8": XL8,
            "XR8": XR8,
            "VL8": VL8,
            "Vblk32": np.ascontiguousarray(V32[ib, :]),
            "UL16": UL16,
            "UR16": np.ascontiguousarray(
                (sdefl * u32[ib])[None, :]).astype(np.float16),
            "U16": np.ascontiguousarray(u32[ib][None, :]).astype(np.float16),
            "BW16": BW16,
        })

    res = run_bass_kernel_spmd(nc, in_maps, core_ids=list(range(NC)),
                               trace=trace)

    out = np.empty((B, N), np.float32)
    for i in range(NC):
        out[:, i * CB:(i + 1) * CB] = res.results[i]["acc_out"].T
    return out, res


def kernel(X, R, coeffs, t_mid, t_half):
    out, _ = _run(X, R, coeffs, t_mid, t_half, trace=False)
    return out


# revision 11
# speedup vs baseline: 2.1282x; 1.3005x over previous
"""ChebyASPIRE spectral filter on 8 TRN2 NeuronCores.

Algorithm (Gram + spectral deflation):
  host prep: Z = X^T X has one dominant eigenpair (lam1, u) -- X is
           iid-sparse-random so Z = strong rank-1 + tight bulk whose
           edge lam2 << lam1.  Host power-iteration (on the fp8-exact
           replica of the device Z) gives (lam1, u) in a handful of
           iterations plus a deflated-power estimate of lam2.  The
           degree-20 Chebyshev filter p restricted to the bulk interval
           [(0-tm)/th, (lam2*SAFETY-tm)/th] is refit with a low-degree
           Chebyshev q (degree adapted to fit error; ~5 here), and the
           deflated u-direction is patched exactly with a rank-1 term
           beta*u*w^T, beta = p(s1) - q(nu), w = u^T V.
  phase 1: Z_defl[:, ib] = X^T X[:, ib] - lam1 u u[ib]^T, column-
           sharded.  fp8 DoubleRow matmuls for X^T X (X pre-scaled by
           32), one extra fp16 rank-1 matmul per 128-row chunk folds
           the deflation into the same PSUM accumulation.  Z_defl fp16.
  phase 2: Chebyshev recurrence on the rescaled operator
           Yhat = (Z_defl - tm' I)/th' (tm' = tm + th*mhat,
           th' = th*hhat) with only deg(q) steps; identical loop
           structure to the degree-20 version: row-sharded matmuls,
           fp8 AllGather of the new t-shard per step, two batch halves
           alternating on the PE to hide the AllGather.  Finally
           acc += u[ib] (beta w)^T via tiny k=1 matmuls.

All DMA-heavy tensors are pre-blocked on the host into partition-major
layouts so every DMA descriptor line is 512B-4KB contiguous.

Inputs come in full; sharding/layout prep happens on host.  Scalars
(tm', th', q coeffs) are baked into the program as immediates; the
program is rebuilt (and NEFF-cached) per distinct scalar set.
"""
import sys

sys.path.insert(0, "/opt/trn_rl_repo")

import numpy as np

M, N, B = 8192, 4096, 256
NC = 8
CB = N // NC          # 512 columns/rows per core
KT1 = M // 128        # 64 k-tiles in phase 1
KP1 = KT1 // 2        # 32 DoubleRow k-pairs in phase 1
MP1 = N // 128        # 32 m-passes in phase 1
KT2 = N // 128        # 32 k-tiles in phase 2
MS2 = CB // 128       # 4 m-subs in phase 2
NH = 2                # batch halves in phase 2
BH = B // NH          # 128 columns per half
XSCALE = 32.0         # host-side fp8 pre-scale on X
ZDESCALE = 1.0 / (XSCALE * XSCALE)

_BUILD_CACHE = {}


def _build(key):
    """Build the SPMD Bass program for (tm', th', deg, *q coeffs)."""
    from concourse import bacc, tile, mybir

    tm, th, deg = key[0], key[1], int(key[2])
    c = key[3:]
    assert len(c) == deg + 1
    f8 = mybir.dt.float8e4
    f16 = mybir.dt.float16
    f32 = mybir.dt.float32
    mult = mybir.AluOpType.mult
    add = mybir.AluOpType.add
    sub = mybir.AluOpType.subtract
    DR = mybir.MatmulPerfMode.DoubleRow

    nc = bacc.Bacc("TRN2", target_bir_lowering=False, debug=False,
                   num_devices=NC)
    # host-blocked layouts (see _run): lines are contiguous per partition
    XL = nc.dram_tensor("XL8", [MP1 * 128 * KT1, 128], f8,
                        kind="ExternalInput")      # [mp, p, kk, mc]
    XR = nc.dram_tensor("XR8", [128 * KT1, CB], f8,
                        kind="ExternalInput")      # [p, kk, cb]
    VL = nc.dram_tensor("VL8", [NH * 128 * KT2, BH], f8,
                        kind="ExternalInput")      # [h, p, kk, b]
    Vb = nc.dram_tensor("Vblk32", [CB, B], f32, kind="ExternalInput")
    # deflation tiles: OU[mp] = lam1 * u[mp chunk] outer u[ib], fp16
    OU = nc.dram_tensor("OU16", [MP1 * 128, CB], f16, kind="ExternalInput")
    # final rank-1 patch: U16 = u[ib], BW16 = beta*w
    U16 = nc.dram_tensor("U16", [1, CB], f16, kind="ExternalInput")
    BW = nc.dram_tensor("BW16", [1, B], f16, kind="ExternalInput")
    acc_out = nc.dram_tensor("acc_out", [CB, B], f32, kind="ExternalOutput")

    RCH = 8                      # k-tiles per rhs_res chunk (phase 1)
    LCH = 16                     # k-tiles per lh chunk (phase 1)
    XR3 = XR[:, :].rearrange("(p kk) cb -> p kk cb", p=128)
    with tile.TileContext(nc) as tc:
        with (
            tc.tile_pool(name="persist", bufs=1) as persist,
            tc.tile_pool(name="lstream", bufs=2) as lstream,
            tc.tile_pool(name="rhsp", bufs=2) as rhsp,
            tc.tile_pool(name="dve", bufs=4) as dvep,
            tc.tile_pool(name="stagep", bufs=2) as stagep,
            tc.tile_pool(name="ps1", bufs=2, space="PSUM") as ps1,
            tc.tile_pool(name="ps2", bufs=6, space="PSUM") as ps2p,
            tc.tile_pool(name="dram", bufs=1, space="DRAM") as dram,
        ):
            # ---------------- phase 1: Z_defl[:, ib] -----------------------
            # resident rhs X[:, ib], chunked so matmuls start early
            rhs_res = [persist.tile([128, RCH, CB], f8, name=f"rhs_res{cc}")
                       for cc in range(KT1 // RCH)]
            for cc in range(KT1 // RCH):
                nc.sync.dma_start(rhs_res[cc][:],
                                  XR3[:, cc * RCH:(cc + 1) * RCH, :])
            OU3 = OU[:, :].rearrange("(mp p) cb -> p mp cb", p=128)

            # phase-2 state init (consumed by DVE only, so these early
            # loads cannot pull matmuls into phase 1)
            tstate = [[persist.tile([128, MS2, BH], f32, name=f"tst{h}_{i}")
                       for i in range(3)] for h in range(NH)]
            acc = [persist.tile([128, MS2, BH], f32, name=f"acc{h}")
                   for h in range(NH)]
            zero = persist.tile([128, BH], f32, name="zero")
            nc.any.memset(zero[:], 0.0)
            Vb3 = Vb[:, :].rearrange("(ms p) b -> p ms b", p=128)
            for h in range(NH):
                nc.sync.dma_start(tstate[h][0][:],
                                  Vb3[:, :, h * BH:(h + 1) * BH])
            u16 = persist.tile([1, CB], f16, name="u16")
            bw = persist.tile([1, B], f16, name="bw")
            nc.sync.dma_start(u16[:], U16[:, :])
            nc.sync.dma_start(bw[:], BW[:, :])

            # warm-up AllGather: burns the first-collective cold cost
            # concurrently with phase-1 compute.
            agin_w = dram.tile([128, MS2 * BH], f8, name="agin_w")
            agout_w = dram.tile([NC * 128, MS2 * BH], f8,
                                addr_space="Shared", name="agout_w")
            nc.gpsimd.collective_compute(
                "AllGather", mybir.AluOpType.bypass,
                replica_groups=[list(range(NC))],
                ins=[agin_w[:]], outs=[agout_w[:]])

            zk = [persist.tile([128, CB], f16, name=f"zk{i}")
                  for i in range(KT2)]

            for mp in range(MP1):
                lhs = [lstream.tile([128, LCH, 128], f8, name=f"lh{h}")
                       for h in range(KT1 // LCH)]
                Xm3 = (XL[mp * 128 * KT1:(mp + 1) * 128 * KT1, :]
                       .rearrange("(p kk) mc -> p kk mc", p=128))
                # lhs triggers on the scalar DGE queue: off the sync
                # queue, and the s==1 rh loads queue up behind them so
                # step-1 matmuls can't be scheduled into phase 1.
                for h in range(KT1 // LCH):
                    nc.scalar.dma_start(lhs[h][:],
                                        Xm3[:, h * LCH:(h + 1) * LCH, :])
                ou = lstream.tile([128, CB], f16, name="ou")
                nc.scalar.dma_start(ou[:], OU3[:, mp, :])
                zps = ps1.tile([128, CB], f32, name="zps")
                for kp in range(KP1):
                    kk = 2 * kp
                    nc.tensor.matmul(
                        zps[:],
                        lhs[kk // LCH][:, kk % LCH:kk % LCH + 2, :],
                        rhs_res[kk // RCH][:, kk % RCH:kk % RCH + 2, :],
                        start=(kp == 0), stop=(kp == KP1 - 1),
                        perf_mode=DR)
                # zk = zps/1024 - lam1 u u^T   (deflation at the copy)
                nc.vector.tensor_scalar_mul(zk[mp][:], zps[:], ZDESCALE)
                nc.vector.tensor_sub(zk[mp][:], zk[mp][:], ou[:])

            # ---------------- phase 2: Chebyshev recurrence ----------------
            agout = [[None] * NH for _ in range(deg)]

            for s in range(1, deg + 1):
                for h in range(NH):
                    # rhs: full t_{s-1} half (4096 x 128) fp8.
                    # SBUF tile [p, kk, b]; global t row = kk*128+p with
                    # kk = 4*rank + ms, matching the blocked agout layout
                    # [rank, p, ms*b] (and VL's [h, p, kk, b]).
                    rh = rhsp.tile([128, NC, MS2, BH], f8, name=f"rh{h}")
                    if s == 1:
                        src = (VL[h * 128 * KT2:(h + 1) * 128 * KT2, :]
                               .rearrange("(p r ms) b -> p r ms b",
                                          p=128, r=NC))
                        nc.scalar.dma_start(rh[:], src[:])
                    else:
                        src = (agout[s - 2][h][:, :]
                               .rearrange("(r p) (ms b) -> p r ms b",
                                          p=128, b=BH))
                        nc.scalar.dma_start(rh[:], src[:])

                    Tc = tstate[h][(s - 1) % 3]
                    Tp = tstate[h][(s - 2) % 3] if s >= 2 else None
                    Tn = tstate[h][s % 3]
                    ach = acc[h]
                    if s < deg:
                        stage = stagep.tile([128, MS2, BH], f8,
                                            name=f"stage{h}")
                        agin = dram.tile([128, MS2 * BH], f8,
                                         name=f"agin{s}_{h}")
                        agin3 = agin[:, :].rearrange("p (ms b) -> p ms b",
                                                     b=BH)

                    for ms in range(MS2):
                        wps = ps2p.tile([128, BH], f32, name="wps")
                        for kk in range(KT2):
                            nc.tensor.matmul(
                                wps[:],
                                zk[kk][:, ms * 128:(ms + 1) * 128],
                                rh[:, kk // MS2, kk % MS2, :],
                                start=(kk == 0), stop=(kk == KT2 - 1))

                        u = dvep.tile([128, BH], f32, name="u")
                        # u = W - tm' * Tc
                        nc.vector.scalar_tensor_tensor(
                            u[:], Tc[:, ms, :], -tm, wps[:],
                            op0=mult, op1=add)
                        if s == 1:
                            # T1 = u / th' ;  acc = q0*V + q1*T1
                            nc.vector.scalar_tensor_tensor(
                                Tn[:, ms, :], u[:], 1.0 / th, zero[:],
                                op0=mult, op1=sub)
                            nc.vector.tensor_scalar_mul(
                                ach[:, ms, :], Tc[:, ms, :], c[0])
                            nc.vector.scalar_tensor_tensor(
                                ach[:, ms, :], Tn[:, ms, :], c[1],
                                ach[:, ms, :], op0=mult, op1=add)
                        else:
                            # Tn = (2/th')*u - Tp ; acc += q_s * Tn
                            nc.vector.scalar_tensor_tensor(
                                Tn[:, ms, :], u[:], 2.0 / th, Tp[:, ms, :],
                                op0=mult, op1=sub)
                        if s < deg:
                            nc.vector.tensor_copy(stage[:, ms, :],
                                                  Tn[:, ms, :])
                        if s > 1:
                            nc.vector.scalar_tensor_tensor(
                                ach[:, ms, :], Tn[:, ms, :], c[s],
                                ach[:, ms, :], op0=mult, op1=add)

                    if s < deg:
                        nc.sync.dma_start(agin3[:], stage[:])
                        agout[s - 1][h] = dram.tile(
                            [NC * 128, MS2 * BH], f8, addr_space="Shared",
                            name=f"agout{s}_{h}")
                        nc.gpsimd.collective_compute(
                            "AllGather",
                            mybir.AluOpType.bypass,
                            replica_groups=[list(range(NC))],
                            ins=[agin[:]],
                            outs=[agout[s - 1][h][:]],
                        )

            # rank-1 patch: acc[h][:, ms, :] += u[ib ms-chunk] (beta w_h)^T
            out3 = acc_out[:, :].rearrange("(ms p) b -> p ms b", p=128)
            for h in range(NH):
                for ms in range(MS2):
                    pr1 = ps2p.tile([128, BH], f32, name="wps")
                    nc.tensor.matmul(
                        pr1[:],
                        u16[:, ms * 128:(ms + 1) * 128],
                        bw[:, h * BH:(h + 1) * BH],
                        start=True, stop=True)
                    nc.vector.scalar_tensor_tensor(
                        acc[h][:, ms, :], pr1[:], 1.0, acc[h][:, ms, :],
                        op0=mult, op1=add)
                nc.sync.dma_start(out3[:, :, h * BH:(h + 1) * BH],
                                  acc[h][:])

    nc.finalize()
    return nc


def _get_program(key):
    key = tuple(np.asarray(key, np.float64).tolist())
    if key not in _BUILD_CACHE:
        _BUILD_CACHE[key] = _build(key)
    return _BUILD_CACHE[key]


def _spectral_prep(X8f, R, coeffs, tm, th):
    """Host-side: eigen structure of the device Z + low-degree refit.

    Returns (key, u32, beta*w) where key = (tm', th', deg, *q).
    """
    N_ = X8f.shape[1]

    def zmv(v):
        return (X8f.T @ (X8f @ v)) * ZDESCALE

    rng = np.random.default_rng(1)
    v = rng.standard_normal(N_).astype(np.float32)
    v /= np.linalg.norm(v)
    lam1 = 0.0
    for _ in range(12):
        w_ = zmv(v)
        lam1 = float(np.linalg.norm(w_))
        v = w_ / lam1
    u = v.astype(np.float64)
    u /= np.linalg.norm(u)

    v2 = rng.standard_normal(N_).astype(np.float32)
    v2 -= (u @ v2).astype(np.float32) * u.astype(np.float32)
    v2 /= np.linalg.norm(v2)
    lam2 = 0.0
    for _ in range(12):
        w_ = zmv(v2)
        w_ -= (u @ w_).astype(np.float32) * u.astype(np.float32)
        lam2 = float(np.linalg.norm(w_))
        v2 = w_ / lam2

    co = np.asarray(coeffs, np.float64)
    DEG0 = len(co) - 1

    def p_eval(x):
        x = np.asarray(x, np.float64)
        t0 = np.ones_like(x)
        t1 = x
        s = co[0] * t0 + co[1] * t1
        for k in range(2, DEG0 + 1):
            t0, t1 = t1, 2 * x * t1 - t0
            s += co[k] * t1
        return s

    s1 = (lam1 - tm) / th
    nu = (0.0 - tm) / th
    SAFETY = 1.35
    hi = (lam2 * SAFETY - tm) / th
    lo = nu
    split_ok = (lam2 * SAFETY < 0.6 * lam1) and hi > lo
    if not split_ok:
        # no spectral gap: fall back to the full interval, degree 20
        lo = nu
        hi = (lam1 * 1.01 - tm) / th
        s1 = hi  # beta -> ~0

    def cheb_fit(lo_, hi_, d_):
        j = np.arange(d_ + 1)
        theta = np.pi * (j + 0.5) / (d_ + 1)
        xn = (lo_ + hi_) / 2 + (hi_ - lo_) / 2 * np.cos(theta)
        fn = p_eval(xn)
        q_ = np.array([2.0 / (d_ + 1) * np.sum(fn * np.cos(k * theta))
                       for k in range(d_ + 1)])
        q_[0] /= 2
        return q_

    xs = np.linspace(lo, hi, 2001)
    deg = DEG0
    for d_ in range(3, DEG0 + 1):
        q = cheb_fit(lo, hi, d_)
        y = (xs - (lo + hi) / 2) / ((hi - lo) / 2)
        t0 = np.ones_like(y)
        t1 = y
        sfit = q[0] * t0 + q[1] * t1
        for k in range(2, d_ + 1):
            t0, t1 = t1, 2 * y * t1 - t0
            sfit += q[k] * t1
        if np.abs(sfit - p_eval(xs)).max() < 2e-5:
            deg = d_
            break
    q = cheb_fit(lo, hi, deg)

    mhat = (lo + hi) / 2
    hhat = (hi - lo) / 2
    tmp = tm + th * mhat
    thp = th * hhat

    # q-sum at the mapped u-eigenvalue (deflated Z has u-eig 0 -> nu)
    ynu = (nu - mhat) / hhat
    t0, t1 = 1.0, ynu
    qnu = q[0] + q[1] * t1
    for k in range(2, deg + 1):
        t0, t1 = t1, 2 * ynu * t1 - t0
        qnu += q[k] * t1
    beta = p_eval(s1) - qnu if split_ok else 0.0

    w = u @ R.T.astype(np.float64)          # (B,)
    key = (tmp, thp, float(deg)) + tuple(q.tolist())
    return key, lam1, u, beta * w, split_ok


def _run(X, R, coeffs, t_mid, t_half, trace=False):
    import ml_dtypes
    from concourse.bass_utils import run_bass_kernel_spmd

    X = np.ascontiguousarray(np.asarray(X, np.float32))
    R = np.ascontiguousarray(np.asarray(R, np.float32))
    coeffs = np.asarray(coeffs, np.float32)
    tm = float(np.asarray(t_mid).reshape(-1)[0])
    th = float(np.asarray(t_half).reshape(-1)[0])

    f8np = ml_dtypes.float8_e4m3
    X8 = (X * XSCALE).astype(f8np)
    X8f = X8.astype(np.float32)
    key, lam1, u, bw, split_ok = _spectral_prep(X8f, R, coeffs, tm, th)

    nc = _get_program(key)

    V32 = np.ascontiguousarray(R.T.astype(np.float32))   # (N, B)
    V8 = V32.astype(f8np)

    # blocked lhs stream [mp, p, kk, mc]: contiguous 2KB lines per (p)
    XL8 = np.ascontiguousarray(
        X8.reshape(KT1, 128, MP1, 128).transpose(2, 1, 0, 3)
    ).reshape(MP1 * 128 * KT1, 128)
    # blocked step-1 rhs [h, p, kk, b]
    VL8 = np.ascontiguousarray(
        V8.reshape(KT2, 128, NH, BH).transpose(2, 1, 0, 3)
    ).reshape(NH * 128 * KT2, BH)

    u32 = u.astype(np.float32)
    lam1_eff = lam1 if split_ok else 0.0
    OU16 = np.ascontiguousarray(
        (lam1_eff * np.outer(u32, u32)).astype(np.float16))  # (N, N) view...
    BW16 = np.ascontiguousarray(bw.astype(np.float16)[None, :])

    in_maps = []
    for i in range(NC):
        ib = slice(i * CB, (i + 1) * CB)
        Xb8 = X8[:, ib]
        # blocked resident rhs [p, kk, cb]: 4KB lines per (p, chunk)
        XR8 = np.ascontiguousarray(
            Xb8.reshape(KT1, 128, CB).transpose(1, 0, 2)
        ).reshape(128 * KT1, CB)
        in_maps.append({
            "XL8": XL8,
            "XR8": XR8,
            "VL8": VL8,
            "Vblk32": np.ascontiguousarray(V32[ib, :]),
            "OU16": np.ascontiguousarray(OU16[:, ib]),
            "U16": np.ascontiguousarray(u32[ib][None, :]).astype(np.float16),
            "BW16": BW16,
        })

    res = run_bass_kernel_spmd(nc, in_maps, core_ids=list(range(NC)),
                               trace=trace)

    out = np.empty((B, N), np.float32)
    for i in range(NC):
        out[:, i * CB:(i + 1) * CB] = res.results[i]["acc_out"].T
    return out, res


def kernel(X, R, coeffs, t_mid, t_half):
    out, _ = _run(X, R, coeffs, t_mid, t_half, trace=False)
    return out


# revision 16
# speedup vs baseline: 2.2211x; 1.0437x over previous
"""ChebyASPIRE spectral filter on 8 TRN2 NeuronCores.

Algorithm (Gram + spectral deflation):
  host prep: Z = X^T X has one dominant eigenpair (lam1, u) -- X is
           iid-sparse-random so Z = strong rank-1 + tight bulk whose
           edge lam2 << lam1.  Host power-iteration (on the fp8-exact
           replica of the device Z) gives (lam1, u) in a handful of
           iterations plus a deflated-power estimate of lam2.  The
           degree-20 Chebyshev filter p restricted to the bulk interval
           [(0-tm)/th, (lam2*SAFETY-tm)/th] is refit with a low-degree
           Chebyshev q (degree adapted to fit error; ~5 here), and the
           deflated u-direction is patched exactly with a rank-1 term
           beta*u*w^T, beta = p(s1) - q(nu), w = u^T V.
  phase 1: Z_defl[:, ib] = X^T X[:, ib] - lam1 u u[ib]^T, column-
           sharded.  fp8 DoubleRow matmuls for X^T X (X pre-scaled by
           32), one extra fp16 rank-1 matmul per 128-row chunk folds
           the deflation into the same PSUM accumulation.  Z_defl fp16.
  phase 2: Chebyshev recurrence on the rescaled operator
           Yhat = (Z_defl - tm' I)/th' (tm' = tm + th*mhat,
           th' = th*hhat) with only deg(q) steps; identical loop
           structure to the degree-20 version: row-sharded matmuls,
           fp8 AllGather of the new t-shard per step, two batch halves
           alternating on the PE to hide the AllGather.  Finally
           acc += u[ib] (beta w)^T via tiny k=1 matmuls.

All DMA-heavy tensors are pre-blocked on the host into partition-major
layouts so every DMA descriptor line is 512B-4KB contiguous.

Inputs come in full; sharding/layout prep happens on host.  Scalars
(tm', th', q coeffs) are baked into the program as immediates; the
program is rebuilt (and NEFF-cached) per distinct scalar set.
"""
import sys

sys.path.insert(0, "/opt/trn_rl_repo")

import numpy as np

M, N, B = 8192, 4096, 256
NC = 8
CB = N // NC          # 512 columns/rows per core
KT1 = M // 128        # 64 k-tiles in phase 1
KP1 = KT1 // 2        # 32 DoubleRow k-pairs in phase 1
MP1 = N // 128        # 32 m-passes in phase 1
KT2 = N // 128        # 32 k-tiles in phase 2
MS2 = CB // 128       # 4 m-subs in phase 2
NH = 2                # batch halves in phase 2
BH = B // NH          # 128 columns per half
XSCALE = 32.0         # host-side fp8 pre-scale on X
ZDESCALE = 1.0 / (XSCALE * XSCALE)

_BUILD_CACHE = {}


def _build(key):
    """Build the SPMD Bass program for (tm', th', deg, *q coeffs)."""
    from concourse import bacc, tile, mybir

    tm, th, deg = key[0], key[1], int(key[2])
    c = key[3:]
    assert len(c) == deg + 1
    f8 = mybir.dt.float8e4
    f16 = mybir.dt.float16
    f32 = mybir.dt.float32
    mult = mybir.AluOpType.mult
    add = mybir.AluOpType.add
    sub = mybir.AluOpType.subtract
    DR = mybir.MatmulPerfMode.DoubleRow

    nc = bacc.Bacc("TRN2", target_bir_lowering=False, debug=False,
                   num_devices=NC)
    # host-blocked layouts (see _run): lines are contiguous per partition
    XL = nc.dram_tensor("XL8", [MP1 * 128 * KT1, 128], f8,
                        kind="ExternalInput")      # [mp, p, kk, mc]
    XR = nc.dram_tensor("XR8", [128 * KT1, CB], f8,
                        kind="ExternalInput")      # [p, kk, cb]
    VL = nc.dram_tensor("VL8", [NH * 128 * KT2, BH], f8,
                        kind="ExternalInput")      # [h, p, kk, b]
    Vb = nc.dram_tensor("Vblk32", [CB, B], f32, kind="ExternalInput")
    # deflation tiles: OU[mp] = lam1 * u[mp chunk] outer u[ib], fp16
    OU = nc.dram_tensor("OU16", [MP1 * 128, CB], f16, kind="ExternalInput")
    # final rank-1 patch: U16 = u[ib], BW16 = beta*w
    U16 = nc.dram_tensor("U16", [1, CB], f16, kind="ExternalInput")
    BW = nc.dram_tensor("BW16", [1, B], f16, kind="ExternalInput")
    acc_out = nc.dram_tensor("acc_out", [CB, B], f32, kind="ExternalOutput")

    RCH = 8                      # k-tiles per rhs_res chunk (phase 1)
    LCH = 16                     # k-tiles per lh chunk (phase 1)
    XR3 = XR[:, :].rearrange("(p kk) cb -> p kk cb", p=128)
    with tile.TileContext(nc) as tc:
        with (
            tc.tile_pool(name="persist", bufs=1) as persist,
            tc.tile_pool(name="lstream", bufs=2) as lstream,
            tc.tile_pool(name="rhsp", bufs=2) as rhsp,
            tc.tile_pool(name="dve", bufs=4) as dvep,
            tc.tile_pool(name="stagep", bufs=2) as stagep,
            tc.tile_pool(name="ps1", bufs=2, space="PSUM") as ps1,
            tc.tile_pool(name="ps2", bufs=6, space="PSUM") as ps2p,
            tc.tile_pool(name="dram", bufs=1, space="DRAM") as dram,
        ):
            # ---------------- phase 1: Z_defl[:, ib] -----------------------
            # resident rhs X[:, ib], chunked so matmuls start early
            rhs_res = [persist.tile([128, RCH, CB], f8, name=f"rhs_res{cc}")
                       for cc in range(KT1 // RCH)]
            for cc in range(KT1 // RCH):
                nc.sync.dma_start(rhs_res[cc][:],
                                  XR3[:, cc * RCH:(cc + 1) * RCH, :])
            OU3 = OU[:, :].rearrange("(mp p) cb -> p mp cb", p=128)

            # phase-2 state init (consumed by DVE only, so these early
            # loads cannot pull matmuls into phase 1)
            tstate = [[persist.tile([128, MS2, BH], f32, name=f"tst{h}_{i}")
                       for i in range(3)] for h in range(NH)]
            acc = [persist.tile([128, MS2, BH], f32, name=f"acc{h}")
                   for h in range(NH)]
            zero = persist.tile([128, BH], f32, name="zero")
            nc.any.memset(zero[:], 0.0)
            Vb3 = Vb[:, :].rearrange("(ms p) b -> p ms b", p=128)
            for h in range(NH):
                nc.sync.dma_start(tstate[h][0][:],
                                  Vb3[:, :, h * BH:(h + 1) * BH])
            u16 = persist.tile([1, CB], f16, name="u16")
            bw = persist.tile([1, B], f16, name="bw")
            nc.sync.dma_start(u16[:], U16[:, :])
            nc.sync.dma_start(bw[:], BW[:, :])

            # warm-up AllGather: burns the first-collective cold cost
            # concurrently with phase-1 compute.
            agin_w = dram.tile([128, MS2 * BH], f8, name="agin_w")
            agout_w = dram.tile([NC * 128, MS2 * BH], f8,
                                addr_space="Shared", name="agout_w")
            nc.gpsimd.collective_compute(
                "AllGather", mybir.AluOpType.bypass,
                replica_groups=[list(range(NC))],
                ins=[agin_w[:]], outs=[agout_w[:]])

            zk = [persist.tile([128, CB], f16, name=f"zk{i}")
                  for i in range(KT2)]

            for mp in range(MP1):
                lhs = [lstream.tile([128, LCH, 128], f8, name=f"lh{h}")
                       for h in range(KT1 // LCH)]
                Xm3 = (XL[mp * 128 * KT1:(mp + 1) * 128 * KT1, :]
                       .rearrange("(p kk) mc -> p kk mc", p=128))
                # lhs triggers on the scalar DGE queue: off the sync
                # queue, and the s==1 rh loads queue up behind them so
                # step-1 matmuls can't be scheduled into phase 1.
                for h in range(KT1 // LCH):
                    nc.scalar.dma_start(lhs[h][:],
                                        Xm3[:, h * LCH:(h + 1) * LCH, :])
                ou = lstream.tile([128, CB], f16, name="ou")
                nc.scalar.dma_start(ou[:], OU3[:, mp, :])
                zps = ps1.tile([128, CB], f32, name="zps")
                for kp in range(KP1):
                    kk = 2 * kp
                    nc.tensor.matmul(
                        zps[:],
                        lhs[kk // LCH][:, kk % LCH:kk % LCH + 2, :],
                        rhs_res[kk // RCH][:, kk % RCH:kk % RCH + 2, :],
                        start=(kp == 0), stop=(kp == KP1 - 1),
                        perf_mode=DR)
                # zk = zps/1024 - lam1 u u^T   (deflation at the copy)
                nc.vector.tensor_scalar_mul(zk[mp][:], zps[:], ZDESCALE)
                nc.vector.tensor_sub(zk[mp][:], zk[mp][:], ou[:])

            # ---------------- phase 2: Chebyshev recurrence ----------------
            agout = [[None] * NH for _ in range(deg)]
            out3 = acc_out[:, :].rearrange("(ms p) b -> p ms b", p=128)

            for s in range(1, deg + 1):
                for h in range(NH):
                    # rhs: full t_{s-1} half (4096 x 128) fp8.
                    # SBUF tile [p, kk, b]; global t row = kk*128+p with
                    # kk = 4*rank + ms, matching the blocked agout layout
                    # [rank, p, ms*b] (and VL's [h, p, kk, b]).
                    rh = rhsp.tile([128, NC, MS2, BH], f8, name=f"rh{h}")
                    if s == 1:
                        src = (VL[h * 128 * KT2:(h + 1) * 128 * KT2, :]
                               .rearrange("(p r ms) b -> p r ms b",
                                          p=128, r=NC))
                        nc.scalar.dma_start(rh[:], src[:])
                    else:
                        src = (agout[s - 2][h][:, :]
                               .rearrange("(r p) (ms b) -> p r ms b",
                                          p=128, b=BH))
                        nc.scalar.dma_start(rh[:], src[:])

                    Tc = tstate[h][(s - 1) % 3]
                    Tp = tstate[h][(s - 2) % 3] if s >= 2 else None
                    Tn = tstate[h][s % 3]
                    ach = acc[h]
                    if s < deg:
                        stage = stagep.tile([128, MS2, BH], f8,
                                            name=f"stage{h}")
                        agin = dram.tile([128, MS2 * BH], f8,
                                         name=f"agin{s}_{h}")
                        agin3 = agin[:, :].rearrange("p (ms b) -> p ms b",
                                                     b=BH)

                    for ms in range(MS2):
                        wps = ps2p.tile([128, BH], f32, name="wps")
                        for kk in range(KT2):
                            nc.tensor.matmul(
                                wps[:],
                                zk[kk][:, ms * 128:(ms + 1) * 128],
                                rh[:, kk // MS2, kk % MS2, :],
                                start=(kk == 0), stop=(kk == KT2 - 1))

                        u = dvep.tile([128, BH], f32, name="u")
                        # u = W - tm' * Tc
                        nc.vector.scalar_tensor_tensor(
                            u[:], Tc[:, ms, :], -tm, wps[:],
                            op0=mult, op1=add)
                        if s == 1:
                            # T1 = u / th' ;  acc = q0*V + q1*T1
                            nc.vector.scalar_tensor_tensor(
                                Tn[:, ms, :], u[:], 1.0 / th, zero[:],
                                op0=mult, op1=sub)
                            nc.vector.tensor_scalar_mul(
                                ach[:, ms, :], Tc[:, ms, :], c[0])
                            nc.vector.scalar_tensor_tensor(
                                ach[:, ms, :], Tn[:, ms, :], c[1],
                                ach[:, ms, :], op0=mult, op1=add)
                        else:
                            # Tn = (2/th')*u - Tp ; acc += q_s * Tn
                            nc.vector.scalar_tensor_tensor(
                                Tn[:, ms, :], u[:], 2.0 / th, Tp[:, ms, :],
                                op0=mult, op1=sub)
                        if s < deg:
                            nc.vector.tensor_copy(stage[:, ms, :],
                                                  Tn[:, ms, :])
                        if s > 1:
                            nc.vector.scalar_tensor_tensor(
                                ach[:, ms, :], Tn[:, ms, :], c[s],
                                ach[:, ms, :], op0=mult, op1=add)

                    if s < deg:
                        nc.sync.dma_start(agin3[:], stage[:])
                        agout[s - 1][h] = dram.tile(
                            [NC * 128, MS2 * BH], f8, addr_space="Shared",
                            name=f"agout{s}_{h}")
                        nc.gpsimd.collective_compute(
                            "AllGather",
                            mybir.AluOpType.bypass,
                            replica_groups=[list(range(NC))],
                            ins=[agin[:]],
                            outs=[agout[s - 1][h][:]],
                        )
                    else:
                        # rank-1 patch + output for this half, right
                        # after its last step (overlaps the other half)
                        for ms in range(MS2):
                            pr1 = ps2p.tile([128, BH], f32, name="wps")
                            nc.tensor.matmul(
                                pr1[:],
                                u16[:, ms * 128:(ms + 1) * 128],
                                bw[:, h * BH:(h + 1) * BH],
                                start=True, stop=True)
                            nc.vector.scalar_tensor_tensor(
                                acc[h][:, ms, :], pr1[:], 1.0,
                                acc[h][:, ms, :], op0=mult, op1=add)
                        nc.sync.dma_start(out3[:, :, h * BH:(h + 1) * BH],
                                          acc[h][:])

    nc.finalize()
    return nc


def _get_program(key):
    key = tuple(np.asarray(key, np.float64).tolist())
    if key not in _BUILD_CACHE:
        _BUILD_CACHE[key] = _build(key)
    return _BUILD_CACHE[key]


def _spectral_prep(X8f, R, coeffs, tm, th):
    """Host-side: eigen structure of the device Z + low-degree refit.

    Returns (key, u32, beta*w) where key = (tm', th', deg, *q).
    """
    N_ = X8f.shape[1]

    def zmv(v):
        return (X8f.T @ (X8f @ v)) * ZDESCALE

    rng = np.random.default_rng(1)
    v = rng.standard_normal(N_).astype(np.float32)
    v /= np.linalg.norm(v)
    lam1 = 0.0
    for _ in range(12):
        w_ = zmv(v)
        lam1 = float(np.linalg.norm(w_))
        v = w_ / lam1
    u = v.astype(np.float64)
    u /= np.linalg.norm(u)

    v2 = rng.standard_normal(N_).astype(np.float32)
    v2 -= (u @ v2).astype(np.float32) * u.astype(np.float32)
    v2 /= np.linalg.norm(v2)
    lam2 = 0.0
    for _ in range(12):
        w_ = zmv(v2)
        w_ -= (u @ w_).astype(np.float32) * u.astype(np.float32)
        lam2 = float(np.linalg.norm(w_))
        v2 = w_ / lam2

    co = np.asarray(coeffs, np.float64)
    DEG0 = len(co) - 1

    def p_eval(x):
        x = np.asarray(x, np.float64)
        t0 = np.ones_like(x)
        t1 = x
        s = co[0] * t0 + co[1] * t1
        for k in range(2, DEG0 + 1):
            t0, t1 = t1, 2 * x * t1 - t0
            s += co[k] * t1
        return s

    s1 = (lam1 - tm) / th
    nu = (0.0 - tm) / th
    SAFETY = 1.35
    hi = (lam2 * SAFETY - tm) / th
    lo = nu
    split_ok = (lam2 * SAFETY < 0.6 * lam1) and hi > lo
    if not split_ok:
        # no spectral gap: fall back to the full interval, degree 20
        lo = nu
        hi = (lam1 * 1.01 - tm) / th
        s1 = hi  # beta -> ~0

    def cheb_fit(lo_, hi_, d_):
        j = np.arange(d_ + 1)
        theta = np.pi * (j + 0.5) / (d_ + 1)
        xn = (lo_ + hi_) / 2 + (hi_ - lo_) / 2 * np.cos(theta)
        fn = p_eval(xn)
        q_ = np.array([2.0 / (d_ + 1) * np.sum(fn * np.cos(k * theta))
                       for k in range(d_ + 1)])
        q_[0] /= 2
        return q_

    xs = np.linspace(lo, hi, 2001)
    deg = DEG0
    for d_ in range(3, DEG0 + 1):
        q = cheb_fit(lo, hi, d_)
        y = (xs - (lo + hi) / 2) / ((hi - lo) / 2)
        t0 = np.ones_like(y)
        t1 = y
        sfit = q[0] * t0 + q[1] * t1
        for k in range(2, d_ + 1):
            t0, t1 = t1, 2 * y * t1 - t0
            sfit += q[k] * t1
        if np.abs(sfit - p_eval(xs)).max() < 2e-5:
            deg = d_
            break
    q = cheb_fit(lo, hi, deg)

    mhat = (lo + hi) / 2
    hhat = (hi - lo) / 2
    tmp = tm + th * mhat
    thp = th * hhat

    # q-sum at the mapped u-eigenvalue (deflated Z has u-eig 0 -> nu)
    ynu = (nu - mhat) / hhat
    t0, t1 = 1.0, ynu
    qnu = q[0] + q[1] * t1
    for k in range(2, deg + 1):
        t0, t1 = t1, 2 * ynu * t1 - t0
        qnu += q[k] * t1
    beta = p_eval(s1) - qnu if split_ok else 0.0

    w = u @ R.T.astype(np.float64)          # (B,)
    key = (tmp, thp, float(deg)) + tuple(q.tolist())
    return key, lam1, u, beta * w, split_ok


def _run(X, R, coeffs, t_mid, t_half, trace=False):
    import ml_dtypes
    from concourse.bass_utils import run_bass_kernel_spmd

    X = np.ascontiguousarray(np.asarray(X, np.float32))
    R = np.ascontiguousarray(np.asarray(R, np.float32))
    coeffs = np.asarray(coeffs, np.float32)
    tm = float(np.asarray(t_mid).reshape(-1)[0])
    th = float(np.asarray(t_half).reshape(-1)[0])

    f8np = ml_dtypes.float8_e4m3
    X8 = (X * XSCALE).astype(f8np)
    X8f = X8.astype(np.float32)
    key, lam1, u, bw, split_ok = _spectral_prep(X8f, R, coeffs, tm, th)

    nc = _get_program(key)

    V32 = np.ascontiguousarray(R.T.astype(np.float32))   # (N, B)
    V8 = V32.astype(f8np)

    # blocked lhs stream [mp, p, kk, mc]: contiguous 2KB lines per (p)
    XL8 = np.ascontiguousarray(
        X8.reshape(KT1, 128, MP1, 128).transpose(2, 1, 0, 3)
    ).reshape(MP1 * 128 * KT1, 128)
    # blocked step-1 rhs [h, p, kk, b]
    VL8 = np.ascontiguousarray(
        V8.reshape(KT2, 128, NH, BH).transpose(2, 1, 0, 3)
    ).reshape(NH * 128 * KT2, BH)

    u32 = u.astype(np.float32)
    lam1_eff = lam1 if split_ok else 0.0
    OU16 = np.ascontiguousarray(
        (lam1_eff * np.outer(u32, u32)).astype(np.float16))  # (N, N) view...
    BW16 = np.ascontiguousarray(bw.astype(np.float16)[None, :])

    in_maps = []
    for i in range(NC):
        ib = slice(i * CB, (i + 1) * CB)
        Xb8 = X8[:, ib]
        # blocked resident rhs [p, kk, cb]: 4KB lines per (p, chunk)
        XR8 = np.ascontiguousarray(
            Xb8.reshape(KT1, 128, CB).transpose(1, 0, 2)
        ).reshape(128 * KT1, CB)
        in_maps.append({
            "XL8": XL8,
            "XR8": XR8,
            "VL8": VL8,
            "Vblk32": np.ascontiguousarray(V32[ib, :]),
            "OU16": np.ascontiguousarray(OU16[:, ib]),
            "U16": np.ascontiguousarray(u32[ib][None, :]).astype(np.float16),
            "BW16": BW16,
        })

    res = run_bass_kernel_spmd(nc, in_maps, core_ids=list(range(NC)),
                               trace=trace)

    out = np.empty((B, N), np.float32)
    for i in range(NC):
        out[:, i * CB:(i + 1) * CB] = res.results[i]["acc_out"].T
    return out, res


def kernel(X, R, coeffs, t_mid, t_half):
    out, _ = _run(X, R, coeffs, t_mid, t_half, trace=False)
    return out
